# revision 27
# baseline (speedup 1.0000x reference)
"""Trainium2 Bass kernel for nn_Encoder_5531917878006 (embedding_lookup).

Computes (embd0, embd1, embd2) of the reference encoder:
  embd0 [8,64,64]     = LN(W_glob) broadcast over batch
  embd1 [8,512,768]   = LN(W_word[tok] + W_type[type] + W_abs)
  embd2 [8,512,512,64]= LN(W_rel[rpe] + diag scatter of (embd1_pre @ W_diag.T + b))

Key structural facts used:
  - LN is applied row-wise over the last dim, and embd2's off-diagonal rows are
    exactly rows of W_rel (130 possible rows) -> LN can be precomputed per table
    row, turning embd2 into a pure 256B-row gather (ids 0..129).
  - The diagonal rows (i,i) are the only ones mixing in diag2; they are computed
    on-device separately ([8,512,64]) and scattered into embd2 on the host.

Sharding: data-parallel over batch B=8 across the 8 NeuronCores (per the hint).
Each core gathers its batch's 512x512 rpe index matrix (host-computed int16
indices, an input to the device program) from the 130-row LN'd table and
streams the 67MB result to HBM with large contiguous DMAs.
"""

import os
import sys
import numpy as np

for _p in ("/root/.axon_site/_ro/trn_rl_repo", "/opt/trn_rl_repo"):
    if os.path.isdir(_p) and _p not in sys.path:
        sys.path.insert(0, _p)

from contextlib import ExitStack

import concourse.bass as bass
import concourse.bacc as bacc
import concourse.mybir as mybir
from concourse import tile
from concourse.bass_utils import run_bass_kernel_spmd

F32 = mybir.dt.float32
I16 = mybir.dt.int16
AF = mybir.ActivationFunctionType
OP = mybir.AluOpType
AX = mybir.AxisListType

B, S = 8, 512
VOCAB, TYPES, D0, D1, D2 = 30522, 2, 64, 768, 64
MAXOFF = 64
GLOB = 64
EPS = 1e-12

NROW = S * S                # 262144 embd2 rows per batch
NPART = 128
NCHUNK = 32                 # output-DMA chunks per core (2MB each)
GSUB = 1                    # sub-gathers per chunk (8192 idxs each; >=16384
                            # crashes the SWDGE ring, 8192 is stable)
CHUNK_C = (NROW // NPART) // NCHUNK     # 64 free-chunks per out-DMA
CSUB = CHUNK_C // GSUB                  # 16 free-chunks per sub-gather
IDX_PER = NPART * CSUB      # 2048 indices per gather
ROWS1_C = S // NPART        # 4 chunks of 128 embd1 rows

# packed consts layout (columns in the [128, CONSTW] consts input)
C_WTD = ROWS1_C * D1                      # 3072
C_TCOL = C_WTD + D1                       # 3840
C_WDT = C_TCOL + ROWS1_C                  # 3844
C_BIAS = C_WDT + (D1 // NPART) * D2       # 4228
C_EYE = C_BIAS + D2                       # 4292
C_WG = C_EYE + NPART                      # 4420
CONSTW = C_WG + D0                        # 4484

# ---- v2 (compute-based embd2) constants ----
JPAD = 64                   # pad cols on each side of j to absorb band spill
SP = S + 2 * JPAD           # padded row width (640)
NOCT = S // 8               # 64 j-octets
NIB = S // NPART            # 4 i-blocks of 128 rows
NDD = 124                   # band diagonals (+-62 excluding 0)
NDG = (NDD + 7) // 8        # 16 dd-groups of 8
MP_PER_TILE = 3             # maskpack octets per SBUF tile at bases {0,32,64}
                            # (PE requires lhsT/rhs base partition in {0,32,64})
NMP = (NOCT + 2) // 3       # 22 tiles (last holds 1 octet)
VB_PER_TILE = 3             # vband dd-groups per SBUF tile at bases {0,32,64}
NVB = (NDG + 2) // 3        # 6 tiles (last holds 1 group)


def _dd_order():
    """SBUF free-dim order of band diagonals: +62..+1 then -1..-62 (so the
    DRAM offset term (-dd*64) is increasing within each half)."""
    return list(range(62, 0, -1)) + list(range(-1, -63, -1))

_CACHED_NC = None


def _rel_pos_ids_np(S_, M):
    r = np.clip(np.arange(S_), None, M - 1)
    c = np.clip(-np.arange(S_), -M + 1, None)
    c = c.copy()
    c[1:] += 2 * M
    d = np.arange(S_)[:, None] - np.arange(S_)[None, :]
    return np.where(d >= 0, c[np.clip(d, 0, S_ - 1)], r[np.clip(-d, 0, S_ - 1)]).astype(
        np.int32
    )


def _ln_rows(x):
    x = np.asarray(x, np.float32)
    m = x.mean(-1, keepdims=True)
    v = ((x - m) ** 2).mean(-1, keepdims=True)
    return ((x - m) / np.sqrt(v + EPS)).astype(np.float32)


def _wrap16(idx, reps=8):
    """dma_gather index layout: [16, n/16] wrapped, replicated to 128 parts."""
    idx = np.asarray(idx, np.int16)
    n = idx.shape[0]
    assert n % 16 == 0
    w = idx.reshape(n // 16, 16).T  # [16, n/16]
    return np.tile(w, (reps, 1))  # [128, n/16]


def _layer_norm(nc, pool, dst, src, parts, n, epscol=None):
    """dst = LN(src) along free dim; src/dst are SBUF APs [parts, n]."""
    m = pool.tile([parts, 1], F32, tag="ln_m")
    nc.vector.reduce_sum(m[:, :], src, axis=AX.X)
    nc.vector.tensor_scalar_mul(m[:, :], m[:, :], 1.0 / n)
    cen = pool.tile([parts, n], F32, tag="ln_cen")
    nc.vector.tensor_scalar_sub(cen[:, :], src, m[:, :])
    sq = pool.tile([parts, n], F32, tag="ln_sq")
    nc.scalar.activation(sq[:, :], cen[:, :], AF.Square)
    v = pool.tile([parts, 1], F32, tag="ln_v")
    nc.vector.reduce_sum(v[:, :], sq[:, :], axis=AX.X)
    sd = pool.tile([parts, 1], F32, tag="ln_sd")
    bias = 0.0 if epscol is None else epscol[:parts, :]
    nc.scalar.activation(sd[:, :], v[:, :], AF.Sqrt, bias=bias, scale=1.0 / n)
    rs = pool.tile([parts, 1], F32, tag="ln_rs")
    nc.vector.reciprocal(rs[:, :], sd[:, :])
    nc.vector.tensor_scalar_mul(dst, cen[:, :], rs[:, :])


def _build_nc():
    nc = bacc.Bacc(None, target_bir_lowering=False, debug=False)

    # -------- inputs (per-core values supplied via in_maps) --------
    rpe_idx = nc.declare_dram_parameter(
        "rpe_idx", [NCHUNK, GSUB, NPART, IDX_PER // 16], I16, isOutput=False
    )
    wrel_ln = nc.declare_dram_parameter("wrel_ln", [130, D2], F32, isOutput=False)
    tok_idx = nc.declare_dram_parameter("tok_idx", [NPART, S // 16], I16, isOutput=False)
    w_word = nc.declare_dram_parameter("w_word", [VOCAB, D1], F32, isOutput=False)
    consts_in = nc.declare_dram_parameter("consts_in", [NPART, CONSTW], F32, isOutput=False)

    # -------- outputs --------
    embd2_out = nc.declare_dram_parameter("embd2_out", [S, S, D2], F32, isOutput=True)
    embd1_out = nc.declare_dram_parameter("embd1_out", [S, D1], F32, isOutput=True)
    diag_out = nc.declare_dram_parameter("diag_out", [S, D2], F32, isOutput=True)
    embd0_out = nc.declare_dram_parameter("embd0_out", [GLOB, D0], F32, isOutput=True)

    with tile.TileContext(nc) as tc, ExitStack() as ctx:
        const = ctx.enter_context(tc.tile_pool(name="const", bufs=1))
        idxp = ctx.enter_context(tc.tile_pool(name="idxp", bufs=1))
        gpool = ctx.enter_context(tc.tile_pool(name="gpool", bufs=2))
        big = ctx.enter_context(tc.tile_pool(name="big", bufs=1))
        work = ctx.enter_context(tc.tile_pool(name="work", bufs=2))
        psp = ctx.enter_context(tc.tile_pool(name="psp", bufs=2, space="PSUM"))

        # ================= embd2: 130-row table gather =================
        nsub = NCHUNK * GSUB
        idx2 = idxp.tile([NPART, nsub * (IDX_PER // 16)], I16)
        nc.sync.dma_start(
            idx2[:, :].rearrange("p (k c) -> p k c", k=nsub),
            rpe_idx[:].rearrange("k g p c -> p (k g) c"),
        )

        run = CHUNK_C * D2  # 4096 f32 per partition per out-chunk
        srun = CSUB * D2    # 1024 f32 per partition per sub-gather
        e2r = (
            embd2_out[:]
            .rearrange("a b c -> (a b c)")
            .rearrange("(p k r) -> p k r", p=NPART, k=NCHUNK)
        )
        for k in range(NCHUNK):
            g = gpool.tile([NPART, run], F32, tag="g2")
            for s in range(GSUB):
                u = k * GSUB + s
                nc.gpsimd.dma_gather(
                    g[:, s * srun : (s + 1) * srun].rearrange(
                        "p (c e) -> p c e", e=D2
                    ),
                    wrel_ln[:],
                    idx2[:, u * (IDX_PER // 16) : (u + 1) * (IDX_PER // 16)],
                    num_idxs=IDX_PER,
                    num_idxs_reg=IDX_PER,
                    elem_size=D2,
                    single_packet=False,
                )
            nc.sync.dma_start(e2r[:, k, :], g[:, :])

        # ================= embd1 + diag =================
        idx1 = idxp.tile([NPART, S // 16], I16)
        nc.sync.dma_start(idx1[:, :], tok_idx[:])
        gw = big.tile([NPART, ROWS1_C * D1], F32, tag="gw")
        nc.gpsimd.dma_gather(
            gw[:, :].rearrange("p (c e) -> p c e", e=D1),
            w_word[:],
            idx1[:, :],
            num_idxs=S,
            num_idxs_reg=S,
            elem_size=D1,
            single_packet=False,
        )

        consts_t = const.tile([NPART, CONSTW], F32)
        nc.sync.dma_start(consts_t[:, :], consts_in[:])
        wabs_t = consts_t[:, 0 : ROWS1_C * D1]
        wtd_t = consts_t[:, C_WTD : C_WTD + D1]
        tcol_t = consts_t[:, C_TCOL : C_TCOL + ROWS1_C]
        wdt_t = consts_t[:, C_WDT : C_WDT + (D1 // NPART) * D2]
        bias_t = consts_t[:, C_BIAS : C_BIAS + D2]
        eye_t = consts_t[:, C_EYE : C_EYE + NPART]
        eps_t = const.tile([NPART, 1], F32)
        nc.vector.memset(eps_t[:, :], float(EPS))

        # X = pre-LN embd1 rows, [128, 4*768]
        x_t = big.tile([NPART, ROWS1_C * D1], F32, tag="x")
        for c in range(ROWS1_C):
            sl = slice(c * D1, (c + 1) * D1)
            # ISA: ops with AP-scalar operands (TensorScalarPtr/STT) get only
            # ONE sync-wait slot, so order ops such that by the time the
            # scalar op runs, all its cross-engine deps are already observed
            # by the DVE vector clock (plain TT ops get 2 slots).
            nc.vector.tensor_copy(x_t[:, sl], gw[:, sl])
            nc.vector.tensor_tensor(x_t[:, sl], x_t[:, sl], wabs_t[:, sl], OP.add)
            nc.vector.scalar_tensor_tensor(
                x_t[:, sl], wtd_t, tcol_t[:, c : c + 1], x_t[:, sl],
                OP.mult, OP.add,
            )

        # diag2 = X @ W_diag.T + (W_rel[0] + b_diag), then LN -> diag rows
        nk = D1 // NPART  # 6 K-chunks
        d_t = big.tile([NPART, ROWS1_C * D2], F32, tag="d")
        for c in range(ROWS1_C):
            xt_s = work.tile([NPART, nk * NPART], F32, tag="xt_s")
            for a in range(nk):
                ps_tr = psp.tile([NPART, NPART], F32, tag="ps_tr")
                nc.tensor.transpose(
                    ps_tr[:, 0:NPART], x_t[:, c * D1 + a * NPART : c * D1 + (a + 1) * NPART],
                    eye_t,
                )
                nc.vector.tensor_copy(xt_s[:, a * NPART : (a + 1) * NPART], ps_tr[:, 0:NPART])
            ps_mm = psp.tile([NPART, D2], F32, tag="ps_mm")
            for a in range(nk):
                nc.tensor.matmul(
                    ps_mm[:, 0:D2],
                    xt_s[:, a * NPART : (a + 1) * NPART],
                    wdt_t[:, a * D2 : (a + 1) * D2],
                    start=(a == 0), stop=(a == nk - 1),
                )
            dsum = work.tile([NPART, D2], F32, tag="dsum")
            nc.vector.scalar_tensor_tensor(
                dsum[:, :], ps_mm[:, 0:D2], 1.0, bias_t, OP.mult, OP.add
            )
            _layer_norm(nc, work, d_t[:, c * D2 : (c + 1) * D2], dsum[:, :], NPART, D2, eps_t)
        nc.sync.dma_start(
            diag_out[:].rearrange("(c p) e -> p c e", c=ROWS1_C),
            d_t[:, :].rearrange("p (c e) -> p c e", e=D2),
        )

        # embd1 = LN(X)
        x1_t = big.tile([NPART, ROWS1_C * D1], F32, tag="x1")
        for c in range(ROWS1_C):
            sl = slice(c * D1, (c + 1) * D1)
            _layer_norm(nc, work, x1_t[:, sl], x_t[:, sl], NPART, D1, eps_t)
        nc.sync.dma_start(
            embd1_out[:].rearrange("(c p) e -> p c e", c=ROWS1_C),
            x1_t[:, :].rearrange("p (c e) -> p c e", e=D1),
        )

        # embd0 = LN(W_glob)
        wg_t = consts_t[0:GLOB, C_WG : C_WG + D0]
        e0_t = work.tile([GLOB, D0], F32, tag="e0")
        _layer_norm(nc, work, e0_t[:, :], wg_t, GLOB, D0, eps_t)
        nc.sync.dma_start(embd0_out[:], e0_t[:, :])

    return nc


def _build_nc2():
    """v2: embd2 generated on-chip via PE matmuls (mask x delta-basis), no
    SWDGE gather for embd2. Output j-dim padded by JPAD on both sides to
    absorb band-edge spill; host slices [:, JPAD:JPAD+S, :].

    Per (i-block 128, j-octet 8): PSUM[128,512] = lhsT[17,128].T @ rhs[17,512]
    where lhsT rows are [m1 x8, m2 x8, ones] (host masks, [j,i]-transposed) and
    rhs rows are [delta(k==j)*K63, delta*K65, C64-tiled]. Band diagonals the
    same way with vband masks and per-diagonal KB rows. PE output is exact
    (each sum has <=2 nonzero f32 terms)."""
    nc = bacc.Bacc(None, target_bir_lowering=False, debug=False)

    tok_idx = nc.declare_dram_parameter("tok_idx", [NPART, S // 16], I16, isOutput=False)
    w_word = nc.declare_dram_parameter("w_word", [VOCAB, D1], F32, isOutput=False)
    consts_in = nc.declare_dram_parameter("consts_in", [NPART, CONSTW], F32, isOutput=False)
    # maskpack: [i-block, tile, 128 rows, 128 i] (3 octets per tile at bases 0/32/64)
    mp_in = nc.declare_dram_parameter("mp_in", [NIB, NMP, NPART, NPART], F32, isOutput=False)
    vb_in = nc.declare_dram_parameter("vb_in", [NIB, NVB, NPART, NPART], F32, isOutput=False)
    rhsb_in = nc.declare_dram_parameter("rhsb_in", [NPART, 8 * D2], F32, isOutput=False)
    rhsd_in = nc.declare_dram_parameter("rhsd_in", [NVB, NPART, 8 * D2], F32, isOutput=False)

    embd2_out = nc.declare_dram_parameter("embd2_out", [S, SP, D2], F32, isOutput=True)
    embd1_out = nc.declare_dram_parameter("embd1_out", [S, D1], F32, isOutput=True)
    diag_out = nc.declare_dram_parameter("diag_out", [S, D2], F32, isOutput=True)
    embd0_out = nc.declare_dram_parameter("embd0_out", [GLOB, D0], F32, isOutput=True)

    with tile.TileContext(nc) as tc, ExitStack() as ctx:
        const = ctx.enter_context(tc.tile_pool(name="const", bufs=1))
        idxp = ctx.enter_context(tc.tile_pool(name="idxp", bufs=1))
        big = ctx.enter_context(tc.tile_pool(name="big", bufs=1))
        work = ctx.enter_context(tc.tile_pool(name="work", bufs=2))
        mpool = ctx.enter_context(tc.tile_pool(name="mpool", bufs=2))
        stage = ctx.enter_context(tc.tile_pool(name="stage", bufs=4))
        psp = ctx.enter_context(tc.tile_pool(name="psp", bufs=2, space="PSUM"))

        # ---- constants ----
        consts_t = const.tile([NPART, CONSTW], F32)
        nc.sync.dma_start(consts_t[:, :], consts_in[:])
        wabs_t = consts_t[:, 0 : ROWS1_C * D1]
        wtd_t = consts_t[:, C_WTD : C_WTD + D1]
        tcol_t = consts_t[:, C_TCOL : C_TCOL + ROWS1_C]
        wdt_t = consts_t[:, C_WDT : C_WDT + (D1 // NPART) * D2]
        bias_t = consts_t[:, C_BIAS : C_BIAS + D2]
        eye_t = consts_t[:, C_EYE : C_EYE + NPART]
        eps_t = const.tile([NPART, 1], F32)
        nc.vector.memset(eps_t[:, :], float(EPS))

        rhsb_t = const.tile([NPART, 8 * D2], F32)
        nc.sync.dma_start(rhsb_t[:, :], rhsb_in[:])
        rhsd_t = []
        for t in range(NVB):
            rt = const.tile([NPART, 8 * D2], F32, tag=f"rhsd{t}")
            nc.sync.dma_start(rt[:, :], rhsd_in[t])
            rhsd_t.append(rt)

        ev = [0]

        def evac(dst, src):
            if ev[0] % 3 == 2:
                nc.scalar.activation(dst, src, AF.Copy)
            else:
                nc.vector.tensor_copy(dst, src)
            ev[0] += 1

        # ---- embd2: per i-block ----
        for q in range(NIB):
            i0 = q * NPART
            # stream this block's mask tiles
            mp_t = []
            for t in range(NMP):
                mt = mpool.tile([NPART, NPART], F32, tag=f"mp{t}")
                nc.sync.dma_start(mt[:, :], mp_in[q, t])
                mp_t.append(mt)
            vb_t = []
            for t in range(NVB):
                vt = mpool.tile([NPART, NPART], F32, tag=f"vb{t}")
                nc.sync.dma_start(vt[:, :], vb_in[q, t])
                vb_t.append(vt)

            base_dmas = []
            for og in range(8):  # 8 octet-groups x 8 octets
                st = stage.tile([NPART, 4096], F32, tag="st")
                for h in range(2):
                    ps = psp.tile([NPART, 2048], F32, tag="ps")
                    for u in range(4):
                        o = og * 8 + h * 4 + u
                        mt = mp_t[o // 3]
                        b0 = 32 * (o % 3)
                        nc.tensor.matmul(
                            ps[:, u * 512 : (u + 1) * 512],
                            mt[b0 : b0 + 17, :],
                            rhsb_t[b0 : b0 + 17, :],
                        )
                    evac(st[:, h * 2048 : (h + 1) * 2048], ps[:, :])
                dst = bass.AP(
                    embd2_out, (i0 * SP + JPAD + og * 64) * D2,
                    [[SP * D2, NPART], [1, 4096]],
                )
                base_dmas.append(nc.sync.dma_start(dst, st[:, :]))

            # band: 16 dd-groups -> 4 psum quads; dd slot a: dd=62-a (a<62),
            # dd=-(a-61) (a>=62); DRAM off(i,dd) = i*SP*D2+(JPAD+i-dd)*D2
            quad_slots = [(0, 32), (32, 32), (64, 32), (96, 28)]
            for gq in range(4):
                s0, ns = quad_slots[gq]
                qt = stage.tile([NPART, 2048], F32, tag="st")
                ps = psp.tile([NPART, 2048], F32, tag="ps")
                for u in range(4):
                    g = gq * 4 + u
                    vt = vb_t[g // 3]
                    rt = rhsd_t[g // 3]
                    b0 = 32 * (g % 3)
                    nc.tensor.matmul(
                        ps[:, u * 512 : (u + 1) * 512],
                        vt[b0 : b0 + 9, :],
                        rt[b0 : b0 + 9, :],
                    )
                evac(qt[:, 0 : ns * D2], ps[:, 0 : ns * D2])
                # DMA slots [s0, s0+ns), split at the dd sign flip (slot 62)
                segs = []
                if s0 < 62:
                    n1 = min(ns, 62 - s0)
                    segs.append((s0, n1))
                    if ns > n1:
                        segs.append((62, ns - n1))
                else:
                    segs.append((s0, ns))
                for (sa, sn) in segs:
                    # slot a -> dd: a<62: dd=62-a -> -dd = a-62 ; a>=62: dd=-(a-61) -> -dd = a-61
                    mdd = (sa - 62) if sa < 62 else (sa - 61)
                    dst = bass.AP(
                        embd2_out,
                        (i0 * SP + JPAD + i0 + mdd) * D2,
                        [[(SP + 1) * D2, NPART], [D2, sn], [1, D2]],
                    )
                    d = nc.sync.dma_start(
                        dst,
                        qt[:, (sa - s0) * D2 : (sa - s0 + sn) * D2].rearrange(
                            "p (a e) -> p a e", e=D2
                        ),
                    )
                    for bd in base_dmas:
                        tile.add_dep_helper(d.ins, bd.ins, reason="band over base")

        # ---- embd1 + diag + embd0 ----
        idx1 = idxp.tile([NPART, S // 16], I16)
        nc.sync.dma_start(idx1[:, :], tok_idx[:])
        gw = big.tile([NPART, ROWS1_C * D1], F32, tag="gw")
        nc.gpsimd.dma_gather(
            gw[:, :].rearrange("p (c e) -> p c e", e=D1),
            w_word[:],
            idx1[:, :],
            num_idxs=S,
            num_idxs_reg=S,
            elem_size=D1,
            single_packet=False,
        )
        x_t = big.tile([NPART, ROWS1_C * D1], F32, tag="x")
        for c in range(ROWS1_C):
            sl = slice(c * D1, (c + 1) * D1)
            nc.vector.tensor_copy(x_t[:, sl], gw[:, sl])
            nc.vector.tensor_tensor(x_t[:, sl], x_t[:, sl], wabs_t[:, sl], OP.add)
            nc.vector.scalar_tensor_tensor(
                x_t[:, sl], wtd_t, tcol_t[:, c : c + 1], x_t[:, sl],
                OP.mult, OP.add,
            )
        nk = D1 // NPART
        d_t = big.tile([NPART, ROWS1_C * D2], F32, tag="d")
        for c in range(ROWS1_C):
            xt_s = work.tile([NPART, nk * NPART], F32, tag="xt_s")
            for a in range(nk):
                ps_tr = psp.tile([NPART, 2048], F32, tag="ps")
                nc.tensor.transpose(
                    ps_tr[:, 0:NPART],
                    x_t[:, c * D1 + a * NPART : c * D1 + (a + 1) * NPART],
                    eye_t,
                )
                nc.vector.tensor_copy(
                    xt_s[:, a * NPART : (a + 1) * NPART], ps_tr[:, 0:NPART]
                )
            ps_mm = psp.tile([NPART, 2048], F32, tag="ps")
            for a in range(nk):
                nc.tensor.matmul(
                    ps_mm[:, 0:D2],
                    xt_s[:, a * NPART : (a + 1) * NPART],
                    wdt_t[:, a * D2 : (a + 1) * D2],
                    start=(a == 0), stop=(a == nk - 1),
                )
            dsum = work.tile([NPART, D2], F32, tag="dsum")
            nc.vector.scalar_tensor_tensor(
                dsum[:, :], ps_mm[:, 0:D2], 1.0, bias_t, OP.mult, OP.add
            )
            _layer_norm(nc, work, d_t[:, c * D2 : (c + 1) * D2], dsum[:, :], NPART, D2, eps_t)
        nc.sync.dma_start(
            diag_out[:].rearrange("(c p) e -> p c e", c=ROWS1_C),
            d_t[:, :].rearrange("p (c e) -> p c e", e=D2),
        )
        x1_t = big.tile([NPART, ROWS1_C * D1], F32, tag="x1")
        for c in range(ROWS1_C):
            sl = slice(c * D1, (c + 1) * D1)
            _layer_norm(nc, work, x1_t[:, sl], x_t[:, sl], NPART, D1, eps_t)
        nc.sync.dma_start(
            embd1_out[:].rearrange("(c p) e -> p c e", c=ROWS1_C),
            x1_t[:, :].rearrange("p (c e) -> p c e", e=D1),
        )
        wg_t = consts_t[0:GLOB, C_WG : C_WG + D0]
        e0_t = work.tile([GLOB, D0], F32, tag="e0")
        _layer_norm(nc, work, e0_t[:, :], wg_t, GLOB, D0, eps_t)
        nc.sync.dma_start(embd0_out[:], e0_t[:, :])

    return nc


def _build_null_nc():
    """Same I/O signature as _build_nc but near-zero work — used by bench.py to
    measure dispatch/transfer overhead so it can be subtracted."""
    nc = bacc.Bacc(None, target_bir_lowering=False, debug=False)
    nc.declare_dram_parameter("rpe_idx", [NCHUNK, GSUB, NPART, IDX_PER // 16], I16, isOutput=False)
    nc.declare_dram_parameter("wrel_ln", [130, D2], F32, isOutput=False)
    nc.declare_dram_parameter("tok_idx", [NPART, S // 16], I16, isOutput=False)
    nc.declare_dram_parameter("w_word", [VOCAB, D1], F32, isOutput=False)
    consts_in = nc.declare_dram_parameter("consts_in", [NPART, CONSTW], F32, isOutput=False)
    embd2_out = nc.declare_dram_parameter("embd2_out", [S, S, D2], F32, isOutput=True)
    embd1_out = nc.declare_dram_parameter("embd1_out", [S, D1], F32, isOutput=True)
    diag_out = nc.declare_dram_parameter("diag_out", [S, D2], F32, isOutput=True)
    embd0_out = nc.declare_dram_parameter("embd0_out", [GLOB, D0], F32, isOutput=True)
    with tile.TileContext(nc) as tc, ExitStack() as ctx:
        pool = ctx.enter_context(tc.tile_pool(name="p", bufs=1))
        t = pool.tile([NPART, D2], F32)
        nc.sync.dma_start(t[:, :], consts_in[:, 0:D2])
        nc.sync.dma_start(embd2_out[0, 0:NPART, :], t[:, :])
        nc.sync.dma_start(embd1_out[0:NPART, 0:D2], t[:, :])
        nc.sync.dma_start(diag_out[0:NPART, :], t[:, :])
        nc.sync.dma_start(embd0_out[:], t[0:GLOB, :])
    return nc


def _host_prep(tok_seq, tok_type_ids, W_word, W_type, W_abs, W_rel, W_glob, W_diag, b_diag):
    tok_seq = np.asarray(tok_seq)
    tt = np.asarray(tok_type_ids)
    W_word = np.asarray(W_word, np.float32)
    W_type = np.asarray(W_type, np.float32)
    W_abs = np.asarray(W_abs, np.float32)
    W_rel = np.asarray(W_rel, np.float32)
    W_glob = np.asarray(W_glob, np.float32)
    W_diag = np.asarray(W_diag, np.float32)
    b_diag = np.asarray(b_diag, np.float32)

    wrel_ln = _ln_rows(W_rel)  # [130, 64]

    T = _rel_pos_ids_np(S, MAXOFF)  # [512, 512]
    consts = np.zeros((NPART, CONSTW), np.float32)
    consts[:, 0 : C_WTD] = (
        W_abs.reshape(ROWS1_C, NPART, D1).transpose(1, 0, 2) + W_type[0]
    ).reshape(NPART, -1)
    consts[:, C_WTD : C_TCOL] = W_type[1] - W_type[0]
    # wdt: row k holds W_diag.T[a*128+k, :] at cols C_WDT + a*64
    wdt = W_diag.T.reshape(D1 // NPART, NPART, D2).transpose(1, 0, 2)
    consts[:, C_WDT : C_BIAS] = wdt.reshape(NPART, -1)
    consts[:, C_BIAS : C_EYE] = W_rel[0] + b_diag
    consts[:, C_EYE : C_WG] = np.eye(NPART, dtype=np.float32)
    consts[0:GLOB, C_WG : C_WG + D0] = W_glob
    shared = {
        "wrel_ln": wrel_ln,
        "w_word": W_word,
    }

    in_maps = []
    for b in range(B):
        t = tt[b].astype(np.int64)
        same = t[:, None] == t[None, :]
        r = np.where(same, T, MAXOFF).astype(np.int64)
        r[0, 1:] = 2 * MAXOFF
        r[1:, 0] = 2 * MAXOFF + 1
        rows = r.reshape(-1)  # [262144], row g = i*512+j
        m = rows.reshape(NPART, NROW // NPART)  # partition p holds rows p*2048+c
        chunks = []
        for k in range(NCHUNK):
            subs = []
            for s in range(GSUB):
                c0 = k * CHUNK_C + s * CSUB
                feed = m[:, c0 : c0 + CSUB].T.reshape(-1)  # [2048]
                subs.append(_wrap16(feed))
            chunks.append(np.stack(subs))
        rpe_idx = np.stack(chunks)  # [32, 4, 128, 128] int16

        im = dict(shared)
        im["rpe_idx"] = rpe_idx
        im["tok_idx"] = _wrap16(tok_seq[b].astype(np.int16))
        cc = consts.copy()
        cc[:, C_TCOL : C_TCOL + ROWS1_C] = (
            tt[b].astype(np.float32).reshape(ROWS1_C, NPART).T
        )
        im["consts_in"] = cc
        in_maps.append(im)
    return in_maps


def _host_prep2(tok_seq, tok_type_ids, W_word, W_type, W_abs, W_rel, W_glob, W_diag, b_diag):
    """Host prep for the v2 (compute-based) kernel."""
    base_maps = _host_prep(
        tok_seq, tok_type_ids, W_word, W_type, W_abs, W_rel, W_glob, W_diag, b_diag
    )
    tt = np.asarray(tok_type_ids)
    wrel_ln = _ln_rows(np.asarray(W_rel, np.float32))
    C64 = wrel_ln[64]
    K63 = wrel_ln[63] - C64
    K65 = wrel_ln[65] - C64
    ddo = _dd_order()
    KB = np.zeros((NDG * 8, D2), np.float32)
    for a, dd in enumerate(ddo):
        KB[a] = (wrel_ln[128 - dd] if dd > 0 else wrel_ln[-dd]) - C64

    # rhs constants (shared across cores)
    rhsb = np.zeros((NPART, 8 * D2), np.float32)
    for ol in range(3):
        for k in range(8):
            rhsb[32 * ol + k, k * D2 : (k + 1) * D2] = K63
            rhsb[32 * ol + 8 + k, k * D2 : (k + 1) * D2] = K65
        rhsb[32 * ol + 16] = np.tile(C64, 8)
    rhsd = np.zeros((NVB, NPART, 8 * D2), np.float32)
    for g in range(NDG):
        t, gl = g // 3, g % 3
        for k in range(8):
            a = 8 * g + k
            if a < NDD:
                rhsd[t, 32 * gl + k, k * D2 : (k + 1) * D2] = KB[a]
        rhsd[t, 32 * gl + 8] = np.tile(C64, 8)

    ii = np.arange(S)
    band_valid = np.zeros((S, NDG * 8), bool)
    jband = np.zeros((S, NDG * 8), np.int64)
    for a, dd in enumerate(ddo):
        j = ii - dd
        ok = (j >= 0) & (j < S)
        band_valid[:, a] = ok
        jband[:, a] = np.clip(j, 0, S - 1)

    in_maps = []
    for b in range(B):
        t = tt[b].astype(np.int64)
        v = (t[:, None] == t[None, :]).astype(np.float32)  # [i, j]
        dj = ii[None, :] - ii[:, None]  # j - i
        m1 = v * (dj >= 63)
        m2 = v * (-dj >= 63)
        vband = v[ii[:, None], jband] * band_valid  # [i, a]

        mp = np.zeros((NIB, NMP, NPART, NPART), np.float32)
        vbp = np.zeros((NIB, NVB, NPART, NPART), np.float32)
        for q in range(NIB):
            i0 = q * NPART
            for o in range(NOCT):
                tile_i, ol = o // 3, o % 3
                for k in range(8):
                    j = 8 * o + k
                    mp[q, tile_i, 32 * ol + k] = m1[i0 : i0 + NPART, j]
                    mp[q, tile_i, 32 * ol + 8 + k] = m2[i0 : i0 + NPART, j]
                mp[q, tile_i, 32 * ol + 16] = 1.0
            for g in range(NDG):
                tile_i, gl = g // 3, g % 3
                for k in range(8):
                    a = 8 * g + k
                    if a < NDD:
                        vbp[q, tile_i, 32 * gl + k] = vband[i0 : i0 + NPART, a]
                vbp[q, tile_i, 32 * gl + 8] = 1.0

        im = {
            "tok_idx": base_maps[b]["tok_idx"],
            "w_word": base_maps[b]["w_word"],
            "consts_in": base_maps[b]["consts_in"],
            "mp_in": mp,
            "vb_in": vbp,
            "rhsb_in": rhsb,
            "rhsd_in": rhsd,
        }
        in_maps.append(im)
    return in_maps


VERSION = 2
_CACHED = {}


def run(inputs, trace=False, version=None, **kw):
    version = VERSION if version is None else version
    if version not in _CACHED:
        _CACHED[version] = _build_nc() if version == 1 else _build_nc2()
    nc = _CACHED[version]
    in_maps = (_host_prep if version == 1 else _host_prep2)(**inputs)
    if not nc.is_finalized():
        nc.finalize()
    res = run_bass_kernel_spmd(nc, in_maps, list(range(B)), trace=trace, **kw)

    if version == 1:
        embd2 = np.stack([res.results[b]["embd2_out"] for b in range(B)])
    else:
        embd2 = np.stack(
            [res.results[b]["embd2_out"][:, JPAD : JPAD + S, :] for b in range(B)]
        )
        wrel_ln = _ln_rows(np.asarray(inputs["W_rel"], np.float32))
        embd2[:, 0, 1:, :] = wrel_ln[2 * MAXOFF]
        embd2[:, 1:, 0, :] = wrel_ln[2 * MAXOFF + 1]
    ar = np.arange(S)
    for b in range(B):
        embd2[b, ar, ar, :] = res.results[b]["diag_out"]
    embd1 = np.stack([res.results[b]["embd1_out"] for b in range(B)])
    embd0 = np.broadcast_to(res.results[0]["embd0_out"], (B, GLOB, D0)).copy()
    return (embd0, embd1, embd2), res


def kernel(**inputs):
    out, _ = run(inputs, trace=False)
    return out


# revision 31
# speedup vs baseline: 1.1815x; 1.1815x over previous
"""Trainium2 Bass kernel for nn_Encoder_5531917878006 (embedding_lookup).

Computes (embd0, embd1, embd2) of the reference encoder:
  embd0 [8,64,64]     = LN(W_glob) broadcast over batch
  embd1 [8,512,768]   = LN(W_word[tok] + W_type[type] + W_abs)
  embd2 [8,512,512,64]= LN(W_rel[rpe] + diag scatter of (embd1_pre @ W_diag.T + b))

Key structural facts used:
  - LN is applied row-wise over the last dim, and embd2's off-diagonal rows are
    exactly rows of W_rel (130 possible rows) -> LN can be precomputed per table
    row, turning embd2 into a pure 256B-row gather (ids 0..129).
  - The diagonal rows (i,i) are the only ones mixing in diag2; they are computed
    on-device separately ([8,512,64]) and scattered into embd2 on the host.

Sharding: data-parallel over batch B=8 across the 8 NeuronCores (per the hint).
Each core gathers its batch's 512x512 rpe index matrix (host-computed int16
indices, an input to the device program) from the 130-row LN'd table and
streams the 67MB result to HBM with large contiguous DMAs.
"""

import os
import sys
import numpy as np

for _p in ("/root/.axon_site/_ro/trn_rl_repo", "/opt/trn_rl_repo"):
    if os.path.isdir(_p) and _p not in sys.path:
        sys.path.insert(0, _p)

from contextlib import ExitStack

import concourse.bass as bass
import concourse.bacc as bacc
import concourse.mybir as mybir
from concourse import tile
from concourse.bass_utils import run_bass_kernel_spmd

F32 = mybir.dt.float32
I16 = mybir.dt.int16
AF = mybir.ActivationFunctionType
OP = mybir.AluOpType
AX = mybir.AxisListType

B, S = 8, 512
VOCAB, TYPES, D0, D1, D2 = 30522, 2, 64, 768, 64
MAXOFF = 64
GLOB = 64
EPS = 1e-12

NROW = S * S                # 262144 embd2 rows per batch
NPART = 128
NCHUNK = 32                 # output-DMA chunks per core (2MB each)
GSUB = 1                    # sub-gathers per chunk (8192 idxs each; >=16384
                            # crashes the SWDGE ring, 8192 is stable)
CHUNK_C = (NROW // NPART) // NCHUNK     # 64 free-chunks per out-DMA
CSUB = CHUNK_C // GSUB                  # 16 free-chunks per sub-gather
IDX_PER = NPART * CSUB      # 2048 indices per gather
ROWS1_C = S // NPART        # 4 chunks of 128 embd1 rows

# packed consts layout (columns in the [128, CONSTW] consts input)
C_WTD = ROWS1_C * D1                      # 3072
C_TCOL = C_WTD + D1                       # 3840
C_WDT = C_TCOL + ROWS1_C                  # 3844
C_BIAS = C_WDT + (D1 // NPART) * D2       # 4228
C_EYE = C_BIAS + D2                       # 4292
C_WG = C_EYE + NPART                      # 4420
CONSTW = C_WG + D0                        # 4484

# ---- v2 (compute-based embd2) constants ----
JPAD = 64                   # pad cols on each side of j to absorb band spill
SP = S + 2 * JPAD           # padded row width (640)
NOCT = S // 8               # 64 j-octets
NIB = S // NPART            # 4 i-blocks of 128 rows
NDD = 124                   # band diagonals (+-62 excluding 0)
NDG = (NDD + 7) // 8        # 16 dd-groups of 8
# v3 (bf16 hi/lo split): base K=34 rows/octet at bases {0,64}; band K=18 at {0,32,64}
MP3_PER = 2
NMP3 = NOCT // MP3_PER      # 32 col-chunks
MP_PER_TILE = 3             # maskpack octets per SBUF tile at bases {0,32,64}
                            # (PE requires lhsT/rhs base partition in {0,32,64})
NMP = (NOCT + 2) // 3       # 22 tiles (last holds 1 octet)
VB_PER_TILE = 3             # vband dd-groups per SBUF tile at bases {0,32,64}
NVB = (NDG + 2) // 3        # 6 tiles (last holds 1 group)


def _dd_order():
    """SBUF free-dim order of band diagonals: +62..+1 then -1..-62 (so the
    DRAM offset term (-dd*64) is increasing within each half)."""
    return list(range(62, 0, -1)) + list(range(-1, -63, -1))

_CACHED_NC = None


def _rel_pos_ids_np(S_, M):
    r = np.clip(np.arange(S_), None, M - 1)
    c = np.clip(-np.arange(S_), -M + 1, None)
    c = c.copy()
    c[1:] += 2 * M
    d = np.arange(S_)[:, None] - np.arange(S_)[None, :]
    return np.where(d >= 0, c[np.clip(d, 0, S_ - 1)], r[np.clip(-d, 0, S_ - 1)]).astype(
        np.int32
    )


def _ln_rows(x):
    x = np.asarray(x, np.float32)
    m = x.mean(-1, keepdims=True)
    v = ((x - m) ** 2).mean(-1, keepdims=True)
    return ((x - m) / np.sqrt(v + EPS)).astype(np.float32)


def _wrap16(idx, reps=8):
    """dma_gather index layout: [16, n/16] wrapped, replicated to 128 parts."""
    idx = np.asarray(idx, np.int16)
    n = idx.shape[0]
    assert n % 16 == 0
    w = idx.reshape(n // 16, 16).T  # [16, n/16]
    return np.tile(w, (reps, 1))  # [128, n/16]


def _layer_norm(nc, pool, dst, src, parts, n, epscol=None):
    """dst = LN(src) along free dim; src/dst are SBUF APs [parts, n]."""
    m = pool.tile([parts, 1], F32, tag="ln_m")
    nc.vector.reduce_sum(m[:, :], src, axis=AX.X)
    nc.vector.tensor_scalar_mul(m[:, :], m[:, :], 1.0 / n)
    cen = pool.tile([parts, n], F32, tag="ln_cen")
    nc.vector.tensor_scalar_sub(cen[:, :], src, m[:, :])
    sq = pool.tile([parts, n], F32, tag="ln_sq")
    nc.scalar.activation(sq[:, :], cen[:, :], AF.Square)
    v = pool.tile([parts, 1], F32, tag="ln_v")
    nc.vector.reduce_sum(v[:, :], sq[:, :], axis=AX.X)
    sd = pool.tile([parts, 1], F32, tag="ln_sd")
    bias = 0.0 if epscol is None else epscol[:parts, :]
    nc.scalar.activation(sd[:, :], v[:, :], AF.Sqrt, bias=bias, scale=1.0 / n)
    rs = pool.tile([parts, 1], F32, tag="ln_rs")
    nc.vector.reciprocal(rs[:, :], sd[:, :])
    nc.vector.tensor_scalar_mul(dst, cen[:, :], rs[:, :])


def _build_nc():
    nc = bacc.Bacc(None, target_bir_lowering=False, debug=False)

    # -------- inputs (per-core values supplied via in_maps) --------
    rpe_idx = nc.declare_dram_parameter(
        "rpe_idx", [NCHUNK, GSUB, NPART, IDX_PER // 16], I16, isOutput=False
    )
    wrel_ln = nc.declare_dram_parameter("wrel_ln", [130, D2], F32, isOutput=False)
    tok_idx = nc.declare_dram_parameter("tok_idx", [NPART, S // 16], I16, isOutput=False)
    w_word = nc.declare_dram_parameter("w_word", [VOCAB, D1], F32, isOutput=False)
    consts_in = nc.declare_dram_parameter("consts_in", [NPART, CONSTW], F32, isOutput=False)

    # -------- outputs --------
    embd2_out = nc.declare_dram_parameter("embd2_out", [S, S, D2], F32, isOutput=True)
    embd1_out = nc.declare_dram_parameter("embd1_out", [S, D1], F32, isOutput=True)
    diag_out = nc.declare_dram_parameter("diag_out", [S, D2], F32, isOutput=True)
    embd0_out = nc.declare_dram_parameter("embd0_out", [GLOB, D0], F32, isOutput=True)

    with tile.TileContext(nc) as tc, ExitStack() as ctx:
        const = ctx.enter_context(tc.tile_pool(name="const", bufs=1))
        idxp = ctx.enter_context(tc.tile_pool(name="idxp", bufs=1))
        gpool = ctx.enter_context(tc.tile_pool(name="gpool", bufs=2))
        big = ctx.enter_context(tc.tile_pool(name="big", bufs=1))
        work = ctx.enter_context(tc.tile_pool(name="work", bufs=2))
        psp = ctx.enter_context(tc.tile_pool(name="psp", bufs=2, space="PSUM"))

        # ================= embd2: 130-row table gather =================
        nsub = NCHUNK * GSUB
        idx2 = idxp.tile([NPART, nsub * (IDX_PER // 16)], I16)
        nc.sync.dma_start(
            idx2[:, :].rearrange("p (k c) -> p k c", k=nsub),
            rpe_idx[:].rearrange("k g p c -> p (k g) c"),
        )

        run = CHUNK_C * D2  # 4096 f32 per partition per out-chunk
        srun = CSUB * D2    # 1024 f32 per partition per sub-gather
        e2r = (
            embd2_out[:]
            .rearrange("a b c -> (a b c)")
            .rearrange("(p k r) -> p k r", p=NPART, k=NCHUNK)
        )
        for k in range(NCHUNK):
            g = gpool.tile([NPART, run], F32, tag="g2")
            for s in range(GSUB):
                u = k * GSUB + s
                nc.gpsimd.dma_gather(
                    g[:, s * srun : (s + 1) * srun].rearrange(
                        "p (c e) -> p c e", e=D2
                    ),
                    wrel_ln[:],
                    idx2[:, u * (IDX_PER // 16) : (u + 1) * (IDX_PER // 16)],
                    num_idxs=IDX_PER,
                    num_idxs_reg=IDX_PER,
                    elem_size=D2,
                    single_packet=False,
                )
            nc.sync.dma_start(e2r[:, k, :], g[:, :])

        # ================= embd1 + diag =================
        idx1 = idxp.tile([NPART, S // 16], I16)
        nc.sync.dma_start(idx1[:, :], tok_idx[:])
        gw = big.tile([NPART, ROWS1_C * D1], F32, tag="gw")
        nc.gpsimd.dma_gather(
            gw[:, :].rearrange("p (c e) -> p c e", e=D1),
            w_word[:],
            idx1[:, :],
            num_idxs=S,
            num_idxs_reg=S,
            elem_size=D1,
            single_packet=False,
        )

        consts_t = const.tile([NPART, CONSTW], F32)
        nc.scalar.dma_start(consts_t[:, :], consts_in[:])
        wabs_t = consts_t[:, 0 : ROWS1_C * D1]
        wtd_t = consts_t[:, C_WTD : C_WTD + D1]
        tcol_t = consts_t[:, C_TCOL : C_TCOL + ROWS1_C]
        wdt_t = consts_t[:, C_WDT : C_WDT + (D1 // NPART) * D2]
        bias_t = consts_t[:, C_BIAS : C_BIAS + D2]
        eye_t = consts_t[:, C_EYE : C_EYE + NPART]
        eps_t = const.tile([NPART, 1], F32)
        nc.vector.memset(eps_t[:, :], float(EPS))

        # X = pre-LN embd1 rows, [128, 4*768]
        x_t = big.tile([NPART, ROWS1_C * D1], F32, tag="x")
        for c in range(ROWS1_C):
            sl = slice(c * D1, (c + 1) * D1)
            # ISA: ops with AP-scalar operands (TensorScalarPtr/STT) get only
            # ONE sync-wait slot, so order ops such that by the time the
            # scalar op runs, all its cross-engine deps are already observed
            # by the DVE vector clock (plain TT ops get 2 slots).
            nc.vector.tensor_copy(x_t[:, sl], gw[:, sl])
            nc.vector.tensor_tensor(x_t[:, sl], x_t[:, sl], wabs_t[:, sl], OP.add)
            nc.vector.scalar_tensor_tensor(
                x_t[:, sl], wtd_t, tcol_t[:, c : c + 1], x_t[:, sl],
                OP.mult, OP.add,
            )

        # diag2 = X @ W_diag.T + (W_rel[0] + b_diag), then LN -> diag rows
        nk = D1 // NPART  # 6 K-chunks
        d_t = big.tile([NPART, ROWS1_C * D2], F32, tag="d")
        for c in range(ROWS1_C):
            xt_s = work.tile([NPART, nk * NPART], F32, tag="xt_s")
            for a in range(nk):
                ps_tr = psp.tile([NPART, NPART], F32, tag="ps_tr")
                nc.tensor.transpose(
                    ps_tr[:, 0:NPART], x_t[:, c * D1 + a * NPART : c * D1 + (a + 1) * NPART],
                    eye_t,
                )
                nc.vector.tensor_copy(xt_s[:, a * NPART : (a + 1) * NPART], ps_tr[:, 0:NPART])
            ps_mm = psp.tile([NPART, D2], F32, tag="ps_mm")
            for a in range(nk):
                nc.tensor.matmul(
                    ps_mm[:, 0:D2],
                    xt_s[:, a * NPART : (a + 1) * NPART],
                    wdt_t[:, a * D2 : (a + 1) * D2],
                    start=(a == 0), stop=(a == nk - 1),
                )
            dsum = work.tile([NPART, D2], F32, tag="dsum")
            nc.vector.scalar_tensor_tensor(
                dsum[:, :], ps_mm[:, 0:D2], 1.0, bias_t, OP.mult, OP.add
            )
            _layer_norm(nc, work, d_t[:, c * D2 : (c + 1) * D2], dsum[:, :], NPART, D2, eps_t)
        nc.sync.dma_start(
            diag_out[:].rearrange("(c p) e -> p c e", c=ROWS1_C),
            d_t[:, :].rearrange("p (c e) -> p c e", e=D2),
        )

        # embd1 = LN(X)
        x1_t = big.tile([NPART, ROWS1_C * D1], F32, tag="x1")
        for c in range(ROWS1_C):
            sl = slice(c * D1, (c + 1) * D1)
            _layer_norm(nc, work, x1_t[:, sl], x_t[:, sl], NPART, D1, eps_t)
        nc.sync.dma_start(
            embd1_out[:].rearrange("(c p) e -> p c e", c=ROWS1_C),
            x1_t[:, :].rearrange("p (c e) -> p c e", e=D1),
        )

        # embd0 = LN(W_glob)
        wg_t = consts_t[0:GLOB, C_WG : C_WG + D0]
        e0_t = work.tile([GLOB, D0], F32, tag="e0")
        _layer_norm(nc, work, e0_t[:, :], wg_t, GLOB, D0, eps_t)
        nc.sync.dma_start(embd0_out[:], e0_t[:, :])

    return nc


def _build_nc2():
    """v2: embd2 generated on-chip via PE matmuls (mask x delta-basis), no
    SWDGE gather for embd2. Output j-dim padded by JPAD on both sides to
    absorb band-edge spill; host slices [:, JPAD:JPAD+S, :].

    Per (i-block 128, j-octet 8): PSUM[128,512] = lhsT[17,128].T @ rhs[17,512]
    where lhsT rows are [m1 x8, m2 x8, ones] (host masks, [j,i]-transposed) and
    rhs rows are [delta(k==j)*K63, delta*K65, C64-tiled]. Band diagonals the
    same way with vband masks and per-diagonal KB rows. PE output is exact
    (each sum has <=2 nonzero f32 terms)."""
    nc = bacc.Bacc(None, target_bir_lowering=False, debug=False)

    tok_idx = nc.declare_dram_parameter("tok_idx", [NPART, S // 16], I16, isOutput=False)
    w_word = nc.declare_dram_parameter("w_word", [VOCAB, D1], F32, isOutput=False)
    consts_in = nc.declare_dram_parameter("consts_in", [NPART, CONSTW], F32, isOutput=False)
    # maskpack: [i-block, 128 rows, tile*128 i-cols] (3 octets per tile at bases 0/32/64)
    mp_in = nc.declare_dram_parameter("mp_in", [NIB, NPART, NMP * NPART], F32, isOutput=False)
    vb_in = nc.declare_dram_parameter("vb_in", [NIB, NPART, NVB * NPART], F32, isOutput=False)
    rhsb_in = nc.declare_dram_parameter("rhsb_in", [NPART, 8 * D2], F32, isOutput=False)
    rhsd_in = nc.declare_dram_parameter("rhsd_in", [NVB, NPART, 8 * D2], F32, isOutput=False)

    embd2_out = nc.declare_dram_parameter("embd2_out", [S, SP, D2], F32, isOutput=True)
    embd1_out = nc.declare_dram_parameter("embd1_out", [S, D1], F32, isOutput=True)
    diag_out = nc.declare_dram_parameter("diag_out", [S, D2], F32, isOutput=True)
    embd0_out = nc.declare_dram_parameter("embd0_out", [GLOB, D0], F32, isOutput=True)

    with tile.TileContext(nc) as tc, ExitStack() as ctx:
        const = ctx.enter_context(tc.tile_pool(name="const", bufs=1))
        idxp = ctx.enter_context(tc.tile_pool(name="idxp", bufs=1))
        big = ctx.enter_context(tc.tile_pool(name="big", bufs=1))
        work = ctx.enter_context(tc.tile_pool(name="work", bufs=2))
        mpool = ctx.enter_context(tc.tile_pool(name="mpool", bufs=2))
        stage = ctx.enter_context(tc.tile_pool(name="stage", bufs=4))
        psp = ctx.enter_context(tc.tile_pool(name="psp", bufs=2, space="PSUM"))

        # ---- constants ----
        consts_t = const.tile([NPART, CONSTW], F32)
        nc.scalar.dma_start(consts_t[:, :], consts_in[:])
        wabs_t = consts_t[:, 0 : ROWS1_C * D1]
        wtd_t = consts_t[:, C_WTD : C_WTD + D1]
        tcol_t = consts_t[:, C_TCOL : C_TCOL + ROWS1_C]
        wdt_t = consts_t[:, C_WDT : C_WDT + (D1 // NPART) * D2]
        bias_t = consts_t[:, C_BIAS : C_BIAS + D2]
        eye_t = consts_t[:, C_EYE : C_EYE + NPART]
        eps_t = const.tile([NPART, 1], F32)
        nc.vector.memset(eps_t[:, :], float(EPS))

        rhsb_t = const.tile([NPART, 8 * D2], F32)
        nc.scalar.dma_start(rhsb_t[:, :], rhsb_in[:])
        rhsd_t = []
        for t in range(NVB):
            rt = const.tile([NPART, 8 * D2], F32, tag=f"rhsd{t}")
            nc.scalar.dma_start(rt[:, :], rhsd_in[t])
            rhsd_t.append(rt)

        ev = [0]

        def evac(dst, src):
            if ev[0] % 3 == 2:
                nc.scalar.activation(dst, src, AF.Copy)
            else:
                nc.vector.tensor_copy(dst, src)
            ev[0] += 1

        # ---- embd2: per i-block ----
        for q in range(NIB):
            i0 = q * NPART
            # stream this block's mask tiles (one DMA each, ACT HWDGE ring so
            # they don't queue behind output writes on the SP ring)
            mpm = mpool.tile([NPART, NMP * NPART], F32, tag="mpm")
            nc.scalar.dma_start(mpm[:, :], mp_in[q])
            vbm = mpool.tile([NPART, NVB * NPART], F32, tag="vbm")
            nc.scalar.dma_start(vbm[:, :], vb_in[q])

            base_dmas = []
            for og in range(8):  # 8 octet-groups x 8 octets
                st = stage.tile([NPART, 4096], F32, tag="st")
                for h in range(2):
                    ps = psp.tile([NPART, 2048], F32, tag="ps")
                    for u in range(4):
                        o = og * 8 + h * 4 + u
                        t0 = (o // 3) * NPART
                        b0 = 32 * (o % 3)
                        nc.tensor.matmul(
                            ps[:, u * 512 : (u + 1) * 512],
                            mpm[b0 : b0 + 17, t0 : t0 + NPART],
                            rhsb_t[b0 : b0 + 17, :],
                        )
                    evac(st[:, h * 2048 : (h + 1) * 2048], ps[:, :])
                dst = bass.AP(
                    embd2_out, (i0 * SP + JPAD + og * 64) * D2,
                    [[SP * D2, NPART], [1, 4096]],
                )
                base_dmas.append(nc.sync.dma_start(dst, st[:, :]))

            # band: 16 dd-groups -> 4 psum quads; dd slot a: dd=62-a (a<62),
            # dd=-(a-61) (a>=62); DRAM off(i,dd) = i*SP*D2+(JPAD+i-dd)*D2
            quad_slots = [(0, 32), (32, 32), (64, 32), (96, 28)]
            for gq in range(4):
                s0, ns = quad_slots[gq]
                qt = stage.tile([NPART, 2048], F32, tag="st")
                ps = psp.tile([NPART, 2048], F32, tag="ps")
                for u in range(4):
                    g = gq * 4 + u
                    t0 = (g // 3) * NPART
                    rt = rhsd_t[g // 3]
                    b0 = 32 * (g % 3)
                    nc.tensor.matmul(
                        ps[:, u * 512 : (u + 1) * 512],
                        vbm[b0 : b0 + 9, t0 : t0 + NPART],
                        rt[b0 : b0 + 9, :],
                    )
                evac(qt[:, 0 : ns * D2], ps[:, 0 : ns * D2])
                # DMA slots [s0, s0+ns), split at the dd sign flip (slot 62)
                segs = []
                if s0 < 62:
                    n1 = min(ns, 62 - s0)
                    segs.append((s0, n1))
                    if ns > n1:
                        segs.append((62, ns - n1))
                else:
                    segs.append((s0, ns))
                for (sa, sn) in segs:
                    # slot a -> dd: a<62: dd=62-a -> -dd = a-62 ; a>=62: dd=-(a-61) -> -dd = a-61
                    mdd = (sa - 62) if sa < 62 else (sa - 61)
                    dst = bass.AP(
                        embd2_out,
                        (i0 * SP + JPAD + i0 + mdd) * D2,
                        [[(SP + 1) * D2, NPART], [D2, sn], [1, D2]],
                    )
                    d = nc.sync.dma_start(
                        dst,
                        qt[:, (sa - s0) * D2 : (sa - s0 + sn) * D2].rearrange(
                            "p (a e) -> p a e", e=D2
                        ),
                    )
                    for bd in base_dmas:
                        tile.add_dep_helper(d.ins, bd.ins, reason="band over base")

        # ---- embd1 + diag + embd0 ----
        idx1 = idxp.tile([NPART, S // 16], I16)
        nc.sync.dma_start(idx1[:, :], tok_idx[:])
        gw = big.tile([NPART, ROWS1_C * D1], F32, tag="gw")
        nc.gpsimd.dma_gather(
            gw[:, :].rearrange("p (c e) -> p c e", e=D1),
            w_word[:],
            idx1[:, :],
            num_idxs=S,
            num_idxs_reg=S,
            elem_size=D1,
            single_packet=False,
        )
        x_t = big.tile([NPART, ROWS1_C * D1], F32, tag="x")
        for c in range(ROWS1_C):
            sl = slice(c * D1, (c + 1) * D1)
            nc.vector.tensor_copy(x_t[:, sl], gw[:, sl])
            nc.vector.tensor_tensor(x_t[:, sl], x_t[:, sl], wabs_t[:, sl], OP.add)
            nc.vector.scalar_tensor_tensor(
                x_t[:, sl], wtd_t, tcol_t[:, c : c + 1], x_t[:, sl],
                OP.mult, OP.add,
            )
        nk = D1 // NPART
        d_t = big.tile([NPART, ROWS1_C * D2], F32, tag="d")
        for c in range(ROWS1_C):
            xt_s = work.tile([NPART, nk * NPART], F32, tag="xt_s")
            for a in range(nk):
                ps_tr = psp.tile([NPART, 2048], F32, tag="ps")
                nc.tensor.transpose(
                    ps_tr[:, 0:NPART],
                    x_t[:, c * D1 + a * NPART : c * D1 + (a + 1) * NPART],
                    eye_t,
                )
                nc.vector.tensor_copy(
                    xt_s[:, a * NPART : (a + 1) * NPART], ps_tr[:, 0:NPART]
                )
            ps_mm = psp.tile([NPART, 2048], F32, tag="ps")
            for a in range(nk):
                nc.tensor.matmul(
                    ps_mm[:, 0:D2],
                    xt_s[:, a * NPART : (a + 1) * NPART],
                    wdt_t[:, a * D2 : (a + 1) * D2],
                    start=(a == 0), stop=(a == nk - 1),
                )
            dsum = work.tile([NPART, D2], F32, tag="dsum")
            nc.vector.scalar_tensor_tensor(
                dsum[:, :], ps_mm[:, 0:D2], 1.0, bias_t, OP.mult, OP.add
            )
            _layer_norm(nc, work, d_t[:, c * D2 : (c + 1) * D2], dsum[:, :], NPART, D2, eps_t)
        nc.sync.dma_start(
            diag_out[:].rearrange("(c p) e -> p c e", c=ROWS1_C),
            d_t[:, :].rearrange("p (c e) -> p c e", e=D2),
        )
        x1_t = big.tile([NPART, ROWS1_C * D1], F32, tag="x1")
        for c in range(ROWS1_C):
            sl = slice(c * D1, (c + 1) * D1)
            _layer_norm(nc, work, x1_t[:, sl], x_t[:, sl], NPART, D1, eps_t)
        nc.sync.dma_start(
            embd1_out[:].rearrange("(c p) e -> p c e", c=ROWS1_C),
            x1_t[:, :].rearrange("p (c e) -> p c e", e=D1),
        )
        wg_t = consts_t[0:GLOB, C_WG : C_WG + D0]
        e0_t = work.tile([GLOB, D0], F32, tag="e0")
        _layer_norm(nc, work, e0_t[:, :], wg_t, GLOB, D0, eps_t)
        nc.sync.dma_start(embd0_out[:], e0_t[:, :])

    return nc


def _build_null_nc():
    """Same I/O signature as _build_nc but near-zero work — used by bench.py to
    measure dispatch/transfer overhead so it can be subtracted."""
    nc = bacc.Bacc(None, target_bir_lowering=False, debug=False)
    nc.declare_dram_parameter("rpe_idx", [NCHUNK, GSUB, NPART, IDX_PER // 16], I16, isOutput=False)
    nc.declare_dram_parameter("wrel_ln", [130, D2], F32, isOutput=False)
    nc.declare_dram_parameter("tok_idx", [NPART, S // 16], I16, isOutput=False)
    nc.declare_dram_parameter("w_word", [VOCAB, D1], F32, isOutput=False)
    consts_in = nc.declare_dram_parameter("consts_in", [NPART, CONSTW], F32, isOutput=False)
    embd2_out = nc.declare_dram_parameter("embd2_out", [S, S, D2], F32, isOutput=True)
    embd1_out = nc.declare_dram_parameter("embd1_out", [S, D1], F32, isOutput=True)
    diag_out = nc.declare_dram_parameter("diag_out", [S, D2], F32, isOutput=True)
    embd0_out = nc.declare_dram_parameter("embd0_out", [GLOB, D0], F32, isOutput=True)
    with tile.TileContext(nc) as tc, ExitStack() as ctx:
        pool = ctx.enter_context(tc.tile_pool(name="p", bufs=1))
        t = pool.tile([NPART, D2], F32)
        nc.sync.dma_start(t[:, :], consts_in[:, 0:D2])
        nc.sync.dma_start(embd2_out[0, 0:NPART, :], t[:, :])
        nc.sync.dma_start(embd1_out[0:NPART, 0:D2], t[:, :])
        nc.sync.dma_start(diag_out[0:NPART, :], t[:, :])
        nc.sync.dma_start(embd0_out[:], t[0:GLOB, :])
    return nc


def _host_prep(tok_seq, tok_type_ids, W_word, W_type, W_abs, W_rel, W_glob, W_diag, b_diag):
    tok_seq = np.asarray(tok_seq)
    tt = np.asarray(tok_type_ids)
    W_word = np.asarray(W_word, np.float32)
    W_type = np.asarray(W_type, np.float32)
    W_abs = np.asarray(W_abs, np.float32)
    W_rel = np.asarray(W_rel, np.float32)
    W_glob = np.asarray(W_glob, np.float32)
    W_diag = np.asarray(W_diag, np.float32)
    b_diag = np.asarray(b_diag, np.float32)

    wrel_ln = _ln_rows(W_rel)  # [130, 64]

    T = _rel_pos_ids_np(S, MAXOFF)  # [512, 512]
    consts = np.zeros((NPART, CONSTW), np.float32)
    consts[:, 0 : C_WTD] = (
        W_abs.reshape(ROWS1_C, NPART, D1).transpose(1, 0, 2) + W_type[0]
    ).reshape(NPART, -1)
    consts[:, C_WTD : C_TCOL] = W_type[1] - W_type[0]
    # wdt: row k holds W_diag.T[a*128+k, :] at cols C_WDT + a*64
    wdt = W_diag.T.reshape(D1 // NPART, NPART, D2).transpose(1, 0, 2)
    consts[:, C_WDT : C_BIAS] = wdt.reshape(NPART, -1)
    consts[:, C_BIAS : C_EYE] = W_rel[0] + b_diag
    consts[:, C_EYE : C_WG] = np.eye(NPART, dtype=np.float32)
    consts[0:GLOB, C_WG : C_WG + D0] = W_glob
    shared = {
        "wrel_ln": wrel_ln,
        "w_word": W_word,
    }

    in_maps = []
    for b in range(B):
        t = tt[b].astype(np.int64)
        same = t[:, None] == t[None, :]
        r = np.where(same, T, MAXOFF).astype(np.int64)
        r[0, 1:] = 2 * MAXOFF
        r[1:, 0] = 2 * MAXOFF + 1
        rows = r.reshape(-1)  # [262144], row g = i*512+j
        m = rows.reshape(NPART, NROW // NPART)  # partition p holds rows p*2048+c
        chunks = []
        for k in range(NCHUNK):
            subs = []
            for s in range(GSUB):
                c0 = k * CHUNK_C + s * CSUB
                feed = m[:, c0 : c0 + CSUB].T.reshape(-1)  # [2048]
                subs.append(_wrap16(feed))
            chunks.append(np.stack(subs))
        rpe_idx = np.stack(chunks)  # [32, 4, 128, 128] int16

        im = dict(shared)
        im["rpe_idx"] = rpe_idx
        im["tok_idx"] = _wrap16(tok_seq[b].astype(np.int16))
        cc = consts.copy()
        cc[:, C_TCOL : C_TCOL + ROWS1_C] = (
            tt[b].astype(np.float32).reshape(ROWS1_C, NPART).T
        )
        im["consts_in"] = cc
        in_maps.append(im)
    return in_maps


def _host_prep2(tok_seq, tok_type_ids, W_word, W_type, W_abs, W_rel, W_glob, W_diag, b_diag):
    """Host prep for the v2 (compute-based) kernel."""
    base_maps = _host_prep(
        tok_seq, tok_type_ids, W_word, W_type, W_abs, W_rel, W_glob, W_diag, b_diag
    )
    tt = np.asarray(tok_type_ids)
    wrel_ln = _ln_rows(np.asarray(W_rel, np.float32))
    C64 = wrel_ln[64]
    K63 = wrel_ln[63] - C64
    K65 = wrel_ln[65] - C64
    ddo = _dd_order()
    KB = np.zeros((NDG * 8, D2), np.float32)
    for a, dd in enumerate(ddo):
        KB[a] = (wrel_ln[128 - dd] if dd > 0 else wrel_ln[-dd]) - C64

    # rhs constants (shared across cores)
    rhsb = np.zeros((NPART, 8 * D2), np.float32)
    for ol in range(3):
        for k in range(8):
            rhsb[32 * ol + k, k * D2 : (k + 1) * D2] = K63
            rhsb[32 * ol + 8 + k, k * D2 : (k + 1) * D2] = K65
        rhsb[32 * ol + 16] = np.tile(C64, 8)
    rhsd = np.zeros((NVB, NPART, 8 * D2), np.float32)
    for g in range(NDG):
        t, gl = g // 3, g % 3
        for k in range(8):
            a = 8 * g + k
            if a < NDD:
                rhsd[t, 32 * gl + k, k * D2 : (k + 1) * D2] = KB[a]
        rhsd[t, 32 * gl + 8] = np.tile(C64, 8)

    ii = np.arange(S)
    band_valid = np.zeros((S, NDG * 8), bool)
    jband = np.zeros((S, NDG * 8), np.int64)
    for a, dd in enumerate(ddo):
        j = ii - dd
        ok = (j >= 0) & (j < S)
        band_valid[:, a] = ok
        jband[:, a] = np.clip(j, 0, S - 1)

    in_maps = []
    for b in range(B):
        t = tt[b].astype(np.int64)
        v = (t[:, None] == t[None, :]).astype(np.float32)  # [i, j]
        dj = ii[None, :] - ii[:, None]  # j - i
        m1 = v * (dj >= 63)
        m2 = v * (-dj >= 63)
        vband = v[ii[:, None], jband] * band_valid  # [i, a]

        mp = np.zeros((NIB, NMP, NPART, NPART), np.float32)
        vbp = np.zeros((NIB, NVB, NPART, NPART), np.float32)
        for q in range(NIB):
            i0 = q * NPART
            for o in range(NOCT):
                tile_i, ol = o // 3, o % 3
                for k in range(8):
                    j = 8 * o + k
                    mp[q, tile_i, 32 * ol + k] = m1[i0 : i0 + NPART, j]
                    mp[q, tile_i, 32 * ol + 8 + k] = m2[i0 : i0 + NPART, j]
                mp[q, tile_i, 32 * ol + 16] = 1.0
            for g in range(NDG):
                tile_i, gl = g // 3, g % 3
                for k in range(8):
                    a = 8 * g + k
                    if a < NDD:
                        vbp[q, tile_i, 32 * gl + k] = vband[i0 : i0 + NPART, a]
                vbp[q, tile_i, 32 * gl + 8] = 1.0

        im = {
            "tok_idx": base_maps[b]["tok_idx"],
            "w_word": base_maps[b]["w_word"],
            "consts_in": base_maps[b]["consts_in"],
            "mp_in": mp.transpose(0, 2, 1, 3).reshape(NIB, NPART, NMP * NPART).copy(),
            "vb_in": vbp.transpose(0, 2, 1, 3).reshape(NIB, NPART, NVB * NPART).copy(),
            "rhsb_in": rhsb,
            "rhsd_in": rhsd,
        }
        in_maps.append(im)
    return in_maps


VERSION = 2
_CACHED = {}


BF16 = mybir.dt.bfloat16


def _build_nc3():
    """v3: like v2 but the embd2 matmuls run in bf16 with hi/lo split
    constants (Dekker): each K-vector row is split K = bf16(K) + bf16(K -
    bf16(K)) and both parts carry the same {0,1} mask rows, accumulating in
    f32 PSUM. Masks and ones are exact in bf16, so products are exact and the
    result matches f32 to ~2^-17 relative. PE runs ~4x faster than fp32."""
    nc = bacc.Bacc(None, target_bir_lowering=False, debug=False)

    tok_idx = nc.declare_dram_parameter("tok_idx", [NPART, S // 16], I16, isOutput=False)
    w_word = nc.declare_dram_parameter("w_word", [VOCAB, D1], F32, isOutput=False)
    consts_in = nc.declare_dram_parameter("consts_in", [NPART, CONSTW], F32, isOutput=False)
    mp_in = nc.declare_dram_parameter("mp_in", [NIB, NPART, NMP3 * NPART], BF16, isOutput=False)
    vb_in = nc.declare_dram_parameter("vb_in", [NIB, NPART, NVB * NPART], BF16, isOutput=False)
    rhsb_in = nc.declare_dram_parameter("rhsb_in", [NPART, 8 * D2], BF16, isOutput=False)
    rhsd_in = nc.declare_dram_parameter("rhsd_in", [NVB, NPART, 8 * D2], BF16, isOutput=False)

    embd2_out = nc.declare_dram_parameter("embd2_out", [S, SP, D2], F32, isOutput=True)
    embd1_out = nc.declare_dram_parameter("embd1_out", [S, D1], F32, isOutput=True)
    diag_out = nc.declare_dram_parameter("diag_out", [S, D2], F32, isOutput=True)
    embd0_out = nc.declare_dram_parameter("embd0_out", [GLOB, D0], F32, isOutput=True)

    with tile.TileContext(nc) as tc, ExitStack() as ctx:
        const = ctx.enter_context(tc.tile_pool(name="const", bufs=1))
        idxp = ctx.enter_context(tc.tile_pool(name="idxp", bufs=1))
        big = ctx.enter_context(tc.tile_pool(name="big", bufs=1))
        work = ctx.enter_context(tc.tile_pool(name="work", bufs=2))
        mpool = ctx.enter_context(tc.tile_pool(name="mpool", bufs=2))
        stage = ctx.enter_context(tc.tile_pool(name="stage", bufs=4))
        psp = ctx.enter_context(tc.tile_pool(name="psp", bufs=2, space="PSUM"))

        consts_t = const.tile([NPART, CONSTW], F32)
        nc.scalar.dma_start(consts_t[:, :], consts_in[:])
        wabs_t = consts_t[:, 0 : ROWS1_C * D1]
        wtd_t = consts_t[:, C_WTD : C_WTD + D1]
        tcol_t = consts_t[:, C_TCOL : C_TCOL + ROWS1_C]
        wdt_t = consts_t[:, C_WDT : C_WDT + (D1 // NPART) * D2]
        bias_t = consts_t[:, C_BIAS : C_BIAS + D2]
        eye_t = consts_t[:, C_EYE : C_EYE + NPART]
        eps_t = const.tile([NPART, 1], F32)
        nc.vector.memset(eps_t[:, :], float(EPS))

        rhsb_t = const.tile([NPART, 8 * D2], BF16)
        nc.scalar.dma_start(rhsb_t[:, :], rhsb_in[:])
        rhsd_t = []
        for t in range(NVB):
            rt = const.tile([NPART, 8 * D2], BF16, tag=f"rhsd{t}")
            nc.scalar.dma_start(rt[:, :], rhsd_in[t])
            rhsd_t.append(rt)

        ev = [0]

        def evac(dst, src):
            if ev[0] % 3 == 2:
                nc.scalar.activation(dst, src, AF.Copy)
            else:
                nc.vector.tensor_copy(dst, src)
            ev[0] += 1

        for q in range(NIB):
            i0 = q * NPART
            mpm = mpool.tile([NPART, NMP3 * NPART], BF16, tag="mpm")
            nc.scalar.dma_start(mpm[:, :], mp_in[q])
            vbm = mpool.tile([NPART, NVB * NPART], BF16, tag="vbm")
            nc.scalar.dma_start(vbm[:, :], vb_in[q])

            base_dmas = []
            for og in range(8):
                st = stage.tile([NPART, 4096], F32, tag="st")
                for h in range(2):
                    ps = psp.tile([NPART, 2048], F32, tag="ps")
                    for u in range(4):
                        o = og * 8 + h * 4 + u
                        t0 = (o // MP3_PER) * NPART
                        b0 = 64 * (o % MP3_PER)
                        nc.tensor.matmul(
                            ps[:, u * 512 : (u + 1) * 512],
                            mpm[b0 : b0 + 34, t0 : t0 + NPART],
                            rhsb_t[b0 : b0 + 34, :],
                        )
                    evac(st[:, h * 2048 : (h + 1) * 2048], ps[:, :])
                dst = bass.AP(
                    embd2_out, (i0 * SP + JPAD + og * 64) * D2,
                    [[SP * D2, NPART], [1, 4096]],
                )
                base_dmas.append(nc.sync.dma_start(dst, st[:, :]))

            quad_slots = [(0, 32), (32, 32), (64, 32), (96, 28)]
            for gq in range(4):
                s0, ns = quad_slots[gq]
                qt = stage.tile([NPART, 2048], F32, tag="st")
                ps = psp.tile([NPART, 2048], F32, tag="ps")
                for u in range(4):
                    g = gq * 4 + u
                    t0 = (g // 3) * NPART
                    rt = rhsd_t[g // 3]
                    b0 = 32 * (g % 3)
                    nc.tensor.matmul(
                        ps[:, u * 512 : (u + 1) * 512],
                        vbm[b0 : b0 + 18, t0 : t0 + NPART],
                        rt[b0 : b0 + 18, :],
                    )
                evac(qt[:, 0 : ns * D2], ps[:, 0 : ns * D2])
                segs = []
                if s0 < 62:
                    n1 = min(ns, 62 - s0)
                    segs.append((s0, n1))
                    if ns > n1:
                        segs.append((62, ns - n1))
                else:
                    segs.append((s0, ns))
                for (sa, sn) in segs:
                    mdd = (sa - 62) if sa < 62 else (sa - 61)
                    dst = bass.AP(
                        embd2_out,
                        (i0 * SP + JPAD + i0 + mdd) * D2,
                        [[(SP + 1) * D2, NPART], [D2, sn], [1, D2]],
                    )
                    d = nc.sync.dma_start(
                        dst,
                        qt[:, (sa - s0) * D2 : (sa - s0 + sn) * D2].rearrange(
                            "p (a e) -> p a e", e=D2
                        ),
                    )
                    for bd in base_dmas:
                        tile.add_dep_helper(d.ins, bd.ins, reason="band over base")

        # ---- embd1 + diag + embd0 (same as v2) ----
        idx1 = idxp.tile([NPART, S // 16], I16)
        nc.sync.dma_start(idx1[:, :], tok_idx[:])
        gw = big.tile([NPART, ROWS1_C * D1], F32, tag="gw")
        nc.gpsimd.dma_gather(
            gw[:, :].rearrange("p (c e) -> p c e", e=D1),
            w_word[:],
            idx1[:, :],
            num_idxs=S,
            num_idxs_reg=S,
            elem_size=D1,
            single_packet=False,
        )
        x_t = big.tile([NPART, ROWS1_C * D1], F32, tag="x")
        for c in range(ROWS1_C):
            sl = slice(c * D1, (c + 1) * D1)
            nc.vector.tensor_copy(x_t[:, sl], gw[:, sl])
            nc.vector.tensor_tensor(x_t[:, sl], x_t[:, sl], wabs_t[:, sl], OP.add)
            nc.vector.scalar_tensor_tensor(
                x_t[:, sl], wtd_t, tcol_t[:, c : c + 1], x_t[:, sl],
                OP.mult, OP.add,
            )
        nk = D1 // NPART
        d_t = big.tile([NPART, ROWS1_C * D2], F32, tag="d")
        for c in range(ROWS1_C):
            xt_s = work.tile([NPART, nk * NPART], F32, tag="xt_s")
            for a in range(nk):
                ps_tr = psp.tile([NPART, 2048], F32, tag="ps")
                nc.tensor.transpose(
                    ps_tr[:, 0:NPART],
                    x_t[:, c * D1 + a * NPART : c * D1 + (a + 1) * NPART],
                    eye_t,
                )
                nc.vector.tensor_copy(
                    xt_s[:, a * NPART : (a + 1) * NPART], ps_tr[:, 0:NPART]
                )
            ps_mm = psp.tile([NPART, 2048], F32, tag="ps")
            for a in range(nk):
                nc.tensor.matmul(
                    ps_mm[:, 0:D2],
                    xt_s[:, a * NPART : (a + 1) * NPART],
                    wdt_t[:, a * D2 : (a + 1) * D2],
                    start=(a == 0), stop=(a == nk - 1),
                )
            dsum = work.tile([NPART, D2], F32, tag="dsum")
            nc.vector.scalar_tensor_tensor(
                dsum[:, :], ps_mm[:, 0:D2], 1.0, bias_t, OP.mult, OP.add
            )
            _layer_norm(nc, work, d_t[:, c * D2 : (c + 1) * D2], dsum[:, :], NPART, D2, eps_t)
        nc.sync.dma_start(
            diag_out[:].rearrange("(c p) e -> p c e", c=ROWS1_C),
            d_t[:, :].rearrange("p (c e) -> p c e", e=D2),
        )
        x1_t = big.tile([NPART, ROWS1_C * D1], F32, tag="x1")
        for c in range(ROWS1_C):
            sl = slice(c * D1, (c + 1) * D1)
            _layer_norm(nc, work, x1_t[:, sl], x_t[:, sl], NPART, D1, eps_t)
        nc.sync.dma_start(
            embd1_out[:].rearrange("(c p) e -> p c e", c=ROWS1_C),
            x1_t[:, :].rearrange("p (c e) -> p c e", e=D1),
        )
        wg_t = consts_t[0:GLOB, C_WG : C_WG + D0]
        e0_t = work.tile([GLOB, D0], F32, tag="e0")
        _layer_norm(nc, work, e0_t[:, :], wg_t, GLOB, D0, eps_t)
        nc.sync.dma_start(embd0_out[:], e0_t[:, :])

    return nc


def _host_prep3(tok_seq, tok_type_ids, W_word, W_type, W_abs, W_rel, W_glob, W_diag, b_diag):
    import ml_dtypes

    bf16 = ml_dtypes.bfloat16
    base_maps = _host_prep(
        tok_seq, tok_type_ids, W_word, W_type, W_abs, W_rel, W_glob, W_diag, b_diag
    )
    tt = np.asarray(tok_type_ids)
    wrel_ln = _ln_rows(np.asarray(W_rel, np.float32))
    C64 = wrel_ln[64]
    K63 = wrel_ln[63] - C64
    K65 = wrel_ln[65] - C64
    ddo = _dd_order()
    KB = np.zeros((NDG * 8, D2), np.float32)
    for a, dd in enumerate(ddo):
        KB[a] = (wrel_ln[128 - dd] if dd > 0 else wrel_ln[-dd]) - C64

    def hilo(x):
        hi = x.astype(bf16)
        lo = (x - hi.astype(np.float32)).astype(bf16)
        return hi, lo

    K63h, K63l = hilo(K63)
    K65h, K65l = hilo(K65)
    C64h, C64l = hilo(C64)
    KBh, KBl = hilo(KB)

    # rhs base: rows [0:34] at base 0 and [64:98] at base 64
    rhsb = np.zeros((NPART, 8 * D2), bf16)
    for ob in (0, 64):
        for k in range(8):
            rhsb[ob + k, k * D2 : (k + 1) * D2] = K63h
            rhsb[ob + 8 + k, k * D2 : (k + 1) * D2] = K65h
            rhsb[ob + 17 + k, k * D2 : (k + 1) * D2] = K63l
            rhsb[ob + 25 + k, k * D2 : (k + 1) * D2] = K65l
        rhsb[ob + 16] = np.tile(C64h, 8)
        rhsb[ob + 33] = np.tile(C64l, 8)
    # rhs band: per group g: rows [b0:b0+18]: [KBh x8, C64h, KBl x8, C64l]
    rhsd = np.zeros((NVB, NPART, 8 * D2), bf16)
    for g in range(NDG):
        t, gl = g // 3, g % 3
        b0 = 32 * gl
        for k in range(8):
            a = 8 * g + k
            if a < NDD:
                rhsd[t, b0 + k, k * D2 : (k + 1) * D2] = KBh[a]
                rhsd[t, b0 + 9 + k, k * D2 : (k + 1) * D2] = KBl[a]
        rhsd[t, b0 + 8] = np.tile(C64h, 8)
        rhsd[t, b0 + 17] = np.tile(C64l, 8)

    ii = np.arange(S)
    band_valid = np.zeros((S, NDG * 8), bool)
    jband = np.zeros((S, NDG * 8), np.int64)
    for a, dd in enumerate(ddo):
        j = ii - dd
        ok = (j >= 0) & (j < S)
        band_valid[:, a] = ok
        jband[:, a] = np.clip(j, 0, S - 1)

    in_maps = []
    for b in range(B):
        t = tt[b].astype(np.int64)
        v = (t[:, None] == t[None, :]).astype(np.float32)
        dj = ii[None, :] - ii[:, None]
        m1 = v * (dj >= 63)
        m2 = v * (-dj >= 63)
        vband = v[ii[:, None], jband] * band_valid

        mp = np.zeros((NIB, NMP3, NPART, NPART), bf16)
        vbp = np.zeros((NIB, NVB, NPART, NPART), bf16)
        for q in range(NIB):
            i0 = q * NPART
            for o in range(NOCT):
                tile_i, ol = o // MP3_PER, o % MP3_PER
                b0 = 64 * ol
                for k in range(8):
                    j = 8 * o + k
                    mp[q, tile_i, b0 + k] = m1[i0 : i0 + NPART, j]
                    mp[q, tile_i, b0 + 8 + k] = m2[i0 : i0 + NPART, j]
                    mp[q, tile_i, b0 + 17 + k] = m1[i0 : i0 + NPART, j]
                    mp[q, tile_i, b0 + 25 + k] = m2[i0 : i0 + NPART, j]
                mp[q, tile_i, b0 + 16] = 1.0
                mp[q, tile_i, b0 + 33] = 1.0
            for g in range(NDG):
                tile_i, gl = g // 3, g % 3
                b0 = 32 * gl
                for k in range(8):
                    a = 8 * g + k
                    if a < NDD:
                        vbp[q, tile_i, b0 + k] = vband[i0 : i0 + NPART, a]
                        vbp[q, tile_i, b0 + 9 + k] = vband[i0 : i0 + NPART, a]
                vbp[q, tile_i, b0 + 8] = 1.0
                vbp[q, tile_i, b0 + 17] = 1.0

        im = {
            "tok_idx": base_maps[b]["tok_idx"],
            "w_word": base_maps[b]["w_word"],
            "consts_in": base_maps[b]["consts_in"],
            "mp_in": mp.transpose(0, 2, 1, 3).reshape(NIB, NPART, NMP3 * NPART).copy(),
            "vb_in": vbp.transpose(0, 2, 1, 3).reshape(NIB, NPART, NVB * NPART).copy(),
            "rhsb_in": rhsb,
            "rhsd_in": rhsd,
        }
        in_maps.append(im)
    return in_maps


def run(inputs, trace=False, version=None, **kw):
    version = VERSION if version is None else version
    if version not in _CACHED:
        _CACHED[version] = {1: _build_nc, 2: _build_nc2, 3: _build_nc3}[version]()
    nc = _CACHED[version]
    in_maps = {1: _host_prep, 2: _host_prep2, 3: _host_prep3}[version](**inputs)
    if not nc.is_finalized():
        nc.finalize()
    res = run_bass_kernel_spmd(nc, in_maps, list(range(B)), trace=trace, **kw)

    if version == 1:
        embd2 = np.stack([res.results[b]["embd2_out"] for b in range(B)])
    else:
        embd2 = np.stack(
            [res.results[b]["embd2_out"][:, JPAD : JPAD + S, :] for b in range(B)]
        )
        wrel_ln = _ln_rows(np.asarray(inputs["W_rel"], np.float32))
        embd2[:, 0, 1:, :] = wrel_ln[2 * MAXOFF]
        embd2[:, 1:, 0, :] = wrel_ln[2 * MAXOFF + 1]
    ar = np.arange(S)
    for b in range(B):
        embd2[b, ar, ar, :] = res.results[b]["diag_out"]
    embd1 = np.stack([res.results[b]["embd1_out"] for b in range(B)])
    embd0 = np.broadcast_to(res.results[0]["embd0_out"], (B, GLOB, D0)).copy()
    return (embd0, embd1, embd2), res


def kernel(**inputs):
    out, _ = run(inputs, trace=False)
    return out
